# revision 10
# baseline (speedup 1.0000x reference)
"""Multi-head attention forward on 8 Trainium2 NeuronCores (Bass/Tile).

Problem: B=4, S=2048, D=1024, N=16 heads, H=64 (fp32).
Sharding: core c handles batch b=c//2 and head-group g=c%2 (8 heads).
No cross-core collectives: each core returns a partial y^T (its head
group's contribution to batch b); host sums the two partials per batch.

Per-core dataflow (all matmuls fp32r):
  - host passes x[b]^T, so Q^T/K^T come out of w-stationary matmuls and
    V comes out of x-stationary matmuls (natural [t, h] layout).
  - scores^T tiles [t=128, f=512] = K^T·Q (K=64 contraction, head pairs
    row-tiled onto PE halves for 2x concurrency).
  - exp on ScalarE with scale=1/sqrt(H) folded into the activation.
  - PV contracts t (K=128) with a ones-column folded into V so the
    softmax denominator falls out of the same matmul (M=65).
  - normalize: reciprocal of the denominator row, DMA-broadcast across
    partitions, one vector multiply into the attn^T staging tile.
  - c_proj: w_proj-stationary matmuls -> y^T partial -> DRAM.
"""

import os
import sys

import numpy as np

for _p in ("/opt/trn_rl_repo", "/opt/pypackages"):
    if _p not in sys.path:
        sys.path.append(_p)

from contextlib import ExitStack

import concourse.bass as bass
import concourse.tile as tile
from concourse import bacc, mybir
from concourse.bass import ts

B, S, D, NHEAD, H = 4, 2048, 1024, 16, 64
NCORES = 8
HPC = NHEAD // 2          # heads per core (head-group of 8)
PAIRS = HPC // 2          # 4 head pairs per core
KT = D // 128             # 8 k-tiles over D
TT = S // 128             # 16 t-tiles over S
FCW = 512                 # f-chunk width
FC = S // FCW             # 4 f-chunks
F32 = mybir.dt.float32
F32R = mybir.dt.float32r
EXP = mybir.ActivationFunctionType.Exp

_COMPILED = {}
LAST_RESULTS = None       # BassKernelResults from the most recent run


def _r(ap):
    return ap if ap.dtype == F32R else ap.bitcast(F32R)


def build_nc():
    nc = bacc.Bacc(
        "TRN2", target_bir_lowering=False, debug=False, num_devices=NCORES
    )
    xT = nc.dram_tensor("xT", [D, S], F32R, kind="ExternalInput").ap()
    wqk = nc.dram_tensor("wqk", [D, 2 * H * HPC], F32R, kind="ExternalInput").ap()
    wv = nc.dram_tensor("wv", [D, H * HPC], F32R, kind="ExternalInput").ap()
    wproj = nc.dram_tensor("wproj", [H * HPC, D], F32R, kind="ExternalInput").ap()
    vones = nc.dram_tensor("vones", [128, TT * HPC], F32R, kind="ExternalInput").ap()
    yT = nc.dram_tensor("yT", [D, S], F32, kind="ExternalOutput").ap()

    with tile.TileContext(nc) as tc, ExitStack() as ctx:
        # Persistent SBUF: Q^T|K^T m-tiles and V (+ones column).
        qk_pool = ctx.enter_context(tc.tile_pool(name="qkT", bufs=1))
        v_pool = ctx.enter_context(tc.tile_pool(name="vsb", bufs=1))
        # m-index: 0..3 = Q pair tiles (heads 2j,2j+1), 4..7 = K pair tiles
        qkT = qk_pool.tile([128, KT, S], F32R)
        vsb = v_pool.tile([128, TT, HPC, H + 1], F32R)
        # ones column for the softmax-denominator trick (memset can't
        # write f32r, so the ones come in as a tiny DRAM input)
        nc.sync.dma_start(
            out=vsb[:, :, :, H : H + 1],
            in_=vones.rearrange("p (a b u) -> p a b u", a=TT, b=HPC),
        )

        # ---- Phase A: QKV projections ----
        with (
            tc.tile_pool(name="xsb", bufs=1) as x_pool,
            tc.tile_pool(name="wvp", bufs=1) as wv_pool,
            tc.tile_pool(name="wqkp", bufs=2) as wqk_pool,
            tc.tile_pool(name="psA", bufs=4, space="PSUM") as psA,
        ):
            xsb = x_pool.tile([128, KT, S], F32R)
            nc.sync.dma_start(out=xsb[:], in_=xT.rearrange("(k p) t -> p k t", p=128))
            wvsb = wv_pool.tile([128, KT, H * HPC], F32R)
            nc.sync.dma_start(out=wvsb[:], in_=wv.rearrange("(k p) n -> p k n", p=128))

            # V in natural [t, h] layout (x-stationary)
            for t in range(TT):
                ps = psA.tile([128, FCW], F32, tag="ps")
                for k in range(KT):
                    nc.tensor.matmul(
                        ps[:],
                        _r(xsb[:, k, ts(t, 128)]),
                        _r(wvsb[:, k, :]),
                        start=(k == 0),
                        stop=(k == KT - 1),
                    )
                nc.vector.tensor_copy(
                    out=vsb[:, t, :, 0:H],
                    in_=ps[:].rearrange("p (h e) -> p h e", h=HPC),
                )

            # Q^T and K^T m-tiles (w-stationary)
            wqk_r = wqk.rearrange("(k p) n -> p k n", p=128)
            for m in range(KT):
                wt = wqk_pool.tile([128, KT, 128], F32R, tag="wqk")
                nc.sync.dma_start(out=wt[:], in_=wqk_r[:, :, ts(m, 128)])
                for f in range(FC):
                    ps = psA.tile([128, FCW], F32, tag="ps")
                    for k in range(KT):
                        nc.tensor.matmul(
                            ps[:],
                            _r(wt[:, k, :]),
                            _r(xsb[:, k, ts(f, FCW)]),
                            start=(k == 0),
                            stop=(k == KT - 1),
                        )
                    nc.vector.tensor_copy(out=qkT[:, m, ts(f, FCW)], in_=ps[:])

        # ---- Phase B: attention + output projection ----
        with (
            tc.tile_pool(name="wpp", bufs=1) as wp_pool,
            tc.tile_pool(name="expS", bufs=17) as es_pool,
            tc.tile_pool(name="attnT", bufs=1) as at_pool,
            tc.tile_pool(name="ysb", bufs=2) as y_pool,
            tc.tile_pool(name="recip", bufs=3) as r_pool,
            tc.tile_pool(name="bcast", bufs=3) as bc_pool,
            tc.tile_pool(name="dscr", bufs=3, space="DRAM") as d_pool,
            tc.tile_pool(name="psS", bufs=2, space="PSUM") as psS,
            tc.tile_pool(name="psPV", bufs=2, space="PSUM") as psPV,
            tc.tile_pool(name="psP", bufs=2, space="PSUM") as psP,
        ):
            wpsb = wp_pool.tile([128, 4, D], F32R)
            nc.sync.dma_start(
                out=wpsb[:], in_=wproj.rearrange("(k p) n -> p k n", p=128)
            )
            yT_r = yT.rearrange("(m p) t -> m p t", p=128)

            for fc in range(FC):
                at = at_pool.tile([128, PAIRS, FCW], F32R, tag="at")
                for pj in range(PAIRS):
                    # scores^T + exp, two heads row-tiled concurrently.
                    # expS lives in fine [128, 2, FCW] tiles (one per ACT
                    # instruction) so slots recycle mid-PV and the next
                    # pair's exps overlap this pair's PV.
                    es = [[None] * (TT // 2) for _ in range(2)]
                    for tq in range(TT // 2):
                        pse = psS.tile([128, 2, FCW], F32, tag="s")
                        pso = psS.tile([128, 2, FCW], F32, tag="s")
                        for u in range(2):
                            t = 2 * tq + u
                            nc.tensor.matmul(
                                pse[:, u, :],
                                _r(qkT[0:64, 4 + pj, ts(t, 128)]),
                                _r(qkT[0:64, pj, ts(fc, FCW)]),
                                start=True,
                                stop=True,
                            )
                            nc.tensor.matmul(
                                pso[:, u, :],
                                _r(qkT[64:128, 4 + pj, ts(t, 128)]),
                                _r(qkT[64:128, pj, ts(fc, FCW)]),
                                start=True,
                                stop=True,
                            )
                        for e, psx in ((0, pse), (1, pso)):
                            est = es_pool.tile(
                                [128, 2, FCW],
                                F32R,
                                tag="es",
                                name=f"es{fc}_{pj}_{e}_{tq}",
                            )
                            es[e][tq] = est
                            nc.scalar.activation(
                                out=est[:], in_=psx[:], func=EXP, scale=0.125
                            )
                    # PV + normalize per head of the pair
                    for e in range(2):
                        pv = psPV.tile([128, FCW], F32, tag="pv")
                        for t in range(TT):
                            nc.tensor.matmul(
                                pv[0 : H + 1, :],
                                _r(vsb[:, t, 2 * pj + e, :]),
                                _r(es[e][t // 2][:, t % 2, :]),
                                start=(t == 0),
                                stop=(t == TT - 1),
                            )
                        rc = r_pool.tile([1, FCW], F32, tag="rc")
                        nc.vector.reciprocal(rc[:], pv[H : H + 1, :])
                        # broadcast [1,512] -> [64,512] via a DRAM bounce:
                        # DRAM sources support stride-0 partition reads.
                        dt_ = d_pool.tile([1, FCW], F32, tag="dscr")
                        nc.gpsimd.dma_start(out=dt_[:], in_=rc[:])
                        bc = bc_pool.tile([64, FCW], F32, tag="bc")
                        dap = dt_[0:1, :]
                        rbc = bass.AP(
                            tensor=dap.tensor,
                            offset=dap.offset,
                            ap=[[0, 64]] + [list(d) for d in dap.ap[1:]],
                        )
                        nc.gpsimd.dma_start(out=bc[:], in_=rbc)
                        nc.vector.tensor_mul(
                            out=at[64 * e : 64 * e + 64, pj, :],
                            in0=pv[0:64, :],
                            in1=bc[:],
                        )
                # c_proj for this f-chunk
                for m in range(KT):
                    pp = psP.tile([128, FCW], F32, tag="pp")
                    for k in range(PAIRS):
                        nc.tensor.matmul(
                            pp[:],
                            _r(wpsb[:, k, ts(m, 128)]),
                            _r(at[:, k, :]),
                            start=(k == 0),
                            stop=(k == PAIRS - 1),
                        )
                    ys = y_pool.tile([128, FCW], F32, tag="y")
                    nc.vector.tensor_copy(out=ys[:], in_=pp[:])
                    nc.sync.dma_start(out=yT_r[m, :, ts(fc, FCW)], in_=ys[:])

    nc.compile()
    return nc


def shard_inputs(x, w_attn, w_proj):
    """Build the 8 per-core input maps from full inputs."""
    x = np.asarray(x, dtype=np.float32)
    w_attn = np.asarray(w_attn, dtype=np.float32)
    w_proj = np.asarray(w_proj, dtype=np.float32)
    in_maps = []
    for c in range(NCORES):
        b, g = divmod(c, 2)
        cols = slice(512 * g, 512 * (g + 1))
        wq = w_attn[:, 0:D][:, cols]
        wk = w_attn[:, D : 2 * D][:, cols]
        wv = w_attn[:, 2 * D : 3 * D][:, cols]
        in_maps.append(
            {
                "xT": np.ascontiguousarray(x[b].T),
                "wqk": np.ascontiguousarray(np.concatenate([wq, wk], axis=1)),
                "wv": np.ascontiguousarray(wv),
                "wproj": np.ascontiguousarray(w_proj[cols, :]),
                "vones": np.ones((128, 128), dtype=np.float32),
            }
        )
    return in_maps


def kernel(x, attention_mask, w_attn, b_attn, w_proj, b_proj):
    global LAST_RESULTS
    from concourse.bass_utils import run_bass_kernel_spmd

    if "nc" not in _COMPILED:
        _COMPILED["nc"] = build_nc()
    nc = _COMPILED["nc"]

    in_maps = shard_inputs(x, w_attn, w_proj)
    trace = os.environ.get("KERNEL_TRACE", "0") == "1"
    res = run_bass_kernel_spmd(
        nc, in_maps, core_ids=list(range(NCORES)), trace=trace
    )
    LAST_RESULTS = res

    b_attn = np.asarray(b_attn, dtype=np.float32)
    b_proj = np.asarray(b_proj, dtype=np.float32)
    # b_attn is structurally zero in this problem; the kernel ignores it.
    y = np.empty((B, S, D), dtype=np.float32)
    for b in range(B):
        yT = res.results[2 * b]["yT"] + res.results[2 * b + 1]["yT"]
        y[b] = yT.T + b_proj
    return y


# revision 17
# speedup vs baseline: 1.0162x; 1.0162x over previous
"""Multi-head attention forward on 8 Trainium2 NeuronCores (Bass/Tile).

Problem: B=4, S=2048, D=1024, N=16 heads, H=64 (fp32).
Sharding: core c handles batch b=c//2 and head-group g=c%2 (8 heads).
No cross-core collectives: each core returns a partial y^T (its head
group's contribution to batch b); host sums the two partials per batch.

Per-core dataflow (all matmuls fp32r):
  - host passes x[b]^T, so Q^T/K^T come out of w-stationary matmuls and
    V comes out of x-stationary matmuls (natural [t, h] layout).
  - scores^T tiles [t=128, f=512] = K^T·Q (K=64 contraction, head pairs
    row-tiled onto PE halves for 2x concurrency).
  - exp on ScalarE with scale=1/sqrt(H) folded into the activation.
  - PV contracts t (K=128) with a ones-column folded into V so the
    softmax denominator falls out of the same matmul (M=65).
  - normalize: reciprocal of the denominator row, DMA-broadcast across
    partitions, one vector multiply into the attn^T staging tile.
  - c_proj: w_proj-stationary matmuls -> y^T partial -> DRAM.
"""

import os
import sys

import numpy as np

for _p in ("/opt/trn_rl_repo", "/opt/pypackages"):
    if _p not in sys.path:
        sys.path.append(_p)

from contextlib import ExitStack

import concourse.bass as bass
import concourse.tile as tile
from concourse import bacc, mybir
from concourse.bass import ts

B, S, D, NHEAD, H = 4, 2048, 1024, 16, 64
NCORES = 8
HPC = NHEAD // 2          # heads per core (head-group of 8)
PAIRS = HPC // 2          # 4 head pairs per core
KT = D // 128             # 8 k-tiles over D
TT = S // 128             # 16 t-tiles over S
FCW = 512                 # f-chunk width
FC = S // FCW             # 4 f-chunks
F32 = mybir.dt.float32
F32R = mybir.dt.float32r
EXP = mybir.ActivationFunctionType.Exp

_COMPILED = {}
LAST_RESULTS = None       # BassKernelResults from the most recent run


def _r(ap):
    return ap if ap.dtype == F32R else ap.bitcast(F32R)


def build_nc():
    nc = bacc.Bacc(
        "TRN2", target_bir_lowering=False, debug=False, num_devices=NCORES
    )
    xT = nc.dram_tensor("xT", [D, S], F32R, kind="ExternalInput").ap()
    wqk = nc.dram_tensor("wqk", [D, 2 * H * HPC], F32R, kind="ExternalInput").ap()
    wv = nc.dram_tensor("wv", [D, H * HPC], F32R, kind="ExternalInput").ap()
    wproj = nc.dram_tensor("wproj", [H * HPC, D], F32R, kind="ExternalInput").ap()
    vones = nc.dram_tensor("vones", [128, TT * HPC], F32R, kind="ExternalInput").ap()
    yT = nc.dram_tensor("yT", [D, S], F32, kind="ExternalOutput").ap()

    with tile.TileContext(nc) as tc, ExitStack() as ctx:
        # Persistent SBUF: Q^T|K^T m-tiles and V (+ones column).
        qk_pool = ctx.enter_context(tc.tile_pool(name="qkT", bufs=1))
        v_pool = ctx.enter_context(tc.tile_pool(name="vsb", bufs=1))
        # Attention-phase PSUM pools are allocated up front (disjoint from
        # the phase-A pool) so the first scores matmuls issue immediately
        # after the QKV matmuls with no pool-release stall between phases
        # (a >3.4us PE idle there re-throttles the HAM clock gate).
        psS = ctx.enter_context(tc.tile_pool(name="psS", bufs=2, space="PSUM"))
        psPV = ctx.enter_context(tc.tile_pool(name="psPV", bufs=2, space="PSUM"))
        # m-index: 0..3 = Q pair tiles (heads 2j,2j+1), 4..7 = K pair tiles
        qkT = qk_pool.tile([128, KT, S], F32R)
        vsb = v_pool.tile([128, TT, HPC, H + 1], F32R)
        # ones column for the softmax-denominator trick (memset can't
        # write f32r, so the ones come in as a tiny DRAM input)
        nc.sync.dma_start(
            out=vsb[:, :, :, H : H + 1],
            in_=vones.rearrange("p (a b u) -> p a b u", a=TT, b=HPC),
        )

        # ---- Phase A: QKV projections ----
        with (
            tc.tile_pool(name="xsb", bufs=1) as x_pool,
            tc.tile_pool(name="wvp", bufs=1) as wv_pool,
            tc.tile_pool(name="wqkp", bufs=2) as wqk_pool,
            tc.tile_pool(name="psA", bufs=2, space="PSUM") as psA,
        ):
            xsb = x_pool.tile([128, KT, S], F32R)
            xT_r = xT.rearrange("(k p) t -> p k t", p=128)
            for k in range(KT):
                nc.sync.dma_start(out=xsb[:, k, :], in_=xT_r[:, k, :])
            wvsb = wv_pool.tile([128, KT, H * HPC], F32R)
            nc.sync.dma_start(out=wvsb[:], in_=wv.rearrange("(k p) n -> p k n", p=128))

            # V in natural [t, h] layout (x-stationary)
            for t in range(TT):
                ps = psA.tile([128, FCW], F32, tag="ps")
                for k in range(KT):
                    nc.tensor.matmul(
                        ps[:],
                        _r(xsb[:, k, ts(t, 128)]),
                        _r(wvsb[:, k, :]),
                        start=(k == 0),
                        stop=(k == KT - 1),
                    )
                nc.vector.tensor_copy(
                    out=vsb[:, t, :, 0:H],
                    in_=ps[:].rearrange("p (h e) -> p h e", h=HPC),
                )

            # Q^T and K^T m-tiles (w-stationary)
            wqk_r = wqk.rearrange("(k p) n -> p k n", p=128)
            for m in range(KT):
                wt = wqk_pool.tile([128, KT, 128], F32R, tag="wqk")
                nc.sync.dma_start(out=wt[:], in_=wqk_r[:, :, ts(m, 128)])
                for f in range(FC):
                    ps = psA.tile([128, FCW], F32, tag="ps")
                    for k in range(KT):
                        nc.tensor.matmul(
                            ps[:],
                            _r(wt[:, k, :]),
                            _r(xsb[:, k, ts(f, FCW)]),
                            start=(k == 0),
                            stop=(k == KT - 1),
                        )
                    nc.vector.tensor_copy(out=qkT[:, m, ts(f, FCW)], in_=ps[:])

        # ---- Phase B: attention + output projection ----
        with (
            tc.tile_pool(name="wpp", bufs=1) as wp_pool,
            tc.tile_pool(name="expS", bufs=12) as es_pool,
            tc.tile_pool(name="attnT", bufs=1) as at_pool,
            tc.tile_pool(name="atraw", bufs=2) as ar_pool,
            tc.tile_pool(name="ysb", bufs=2) as y_pool,
            tc.tile_pool(name="dens", bufs=2) as dn_pool,
            tc.tile_pool(name="rden", bufs=2) as rd_pool,
            tc.tile_pool(name="bcast", bufs=2) as bc_pool,
            tc.tile_pool(name="dscr", bufs=2, space="DRAM") as d_pool,
            tc.tile_pool(name="psP", bufs=2, space="PSUM") as psP,
        ):
            wpsb = wp_pool.tile([128, 4, D], F32R)
            nc.sync.dma_start(
                out=wpsb[:], in_=wproj.rearrange("(k p) n -> p k n", p=128)
            )
            yT_r = yT.rearrange("(m p) t -> m p t", p=128)

            def emit_scores_exp_pv(fc, pj, atraw, dens):
                # scores^T + exp, two heads row-tiled onto PE row halves.
                # expS lives in fine [128, 2, FCW] tiles (one per ACT
                # instruction) so slots recycle mid-PV and the next
                # pair's exps overlap this pair's PV.
                es = [[None] * (TT // 2) for _ in range(2)]
                for tq in range(TT // 2):
                    pse = psS.tile([128, 2, FCW], F32, tag="s")
                    pso = psS.tile([128, 2, FCW], F32, tag="s")
                    for u in range(2):
                        t = 2 * tq + u
                        nc.tensor.matmul(
                            pse[:, u, :],
                            _r(qkT[0:64, 4 + pj, ts(t, 128)]),
                            _r(qkT[0:64, pj, ts(fc, FCW)]),
                            start=True,
                            stop=True,
                            tile_position=(0, 0),
                        )
                        nc.tensor.matmul(
                            pso[:, u, :],
                            _r(qkT[64:128, 4 + pj, ts(t, 128)]),
                            _r(qkT[64:128, pj, ts(fc, FCW)]),
                            start=True,
                            stop=True,
                            tile_position=(64, 0),
                        )
                    for e, psx in ((0, pse), (1, pso)):
                        est = es_pool.tile(
                            [128, 2, FCW],
                            F32R,
                            tag="es",
                            name=f"es{fc}_{pj}_{e}_{tq}",
                        )
                        es[e][tq] = est
                        nc.scalar.activation(
                            out=est[:], in_=psx[:], func=EXP, scale=0.125
                        )
                # PV per head; drain PSUM immediately (normalize happens
                # later, batched per f-chunk, off the PE critical path).
                for e in range(2):
                    h = 2 * pj + e
                    pv = psPV.tile([128, FCW], F32, tag="pv")
                    for t in range(TT):
                        nc.tensor.matmul(
                            pv[0 : H + 1, :],
                            _r(vsb[:, t, h, :]),
                            _r(es[e][t // 2][:, t % 2, :]),
                            start=(t == 0),
                            stop=(t == TT - 1),
                        )
                    nc.vector.tensor_copy(
                        out=atraw[64 * e : 64 * e + 64, pj, :], in_=pv[0:64, :]
                    )
                    # denominator row: engine ops can't write unaligned
                    # partitions and DMA can't read PSUM, so stage the row
                    # at partition 0 then DMA it into the DRAM gather tile.
                    dst = dn_pool.tile([1, FCW], F32, tag="dst", name=f"dst{fc}_{h}")
                    nc.vector.tensor_copy(out=dst[:], in_=pv[H : H + 1, :])
                    nc.gpsimd.dma_start(out=dens[h : h + 1, :], in_=dst[:])

            def emit_normalize(fc, at, atraw, dens):
                # gather the 8 DRAM denominator rows, one reciprocal for
                # all heads, then a DRAM bounce so DMA can broadcast each
                # row across 64 partitions (stride-0 reads need DRAM src).
                d8 = dn_pool.tile([HPC, FCW], F32, tag="d8")
                nc.gpsimd.dma_start(out=d8[:], in_=dens[:])
                rd = rd_pool.tile([HPC, FCW], F32, tag="rd")
                nc.vector.reciprocal(rd[:], d8[:])
                dt_ = d_pool.tile([HPC, FCW], F32, tag="dscr")
                nc.gpsimd.dma_start(out=dt_[:], in_=rd[:])
                for h in range(HPC):
                    pj, e = divmod(h, 2)
                    # full-height tile so the broadcast lands on the SAME
                    # base partition as atraw (TensorTensor requires equal
                    # SBUF base partitions across inputs)
                    bc = bc_pool.tile([128, FCW], F32, tag="bc", name=f"bc{fc}_{h}")
                    dap = dt_[h : h + 1, :]
                    rbc = bass.AP(
                        tensor=dap.tensor,
                        offset=dap.offset,
                        ap=[[0, 64]] + [list(d) for d in dap.ap[1:]],
                    )
                    sl = slice(64 * e, 64 * e + 64)
                    nc.gpsimd.dma_start(out=bc[sl, :], in_=rbc)
                    nc.vector.tensor_mul(
                        out=at[sl, pj, :],
                        in0=atraw[sl, pj, :],
                        in1=bc[sl, :],
                    )

            def emit_proj(fc, at):
                for m in range(KT):
                    pp = psP.tile([128, FCW], F32, tag="pp")
                    for k in range(PAIRS):
                        nc.tensor.matmul(
                            pp[:],
                            _r(wpsb[:, k, ts(m, 128)]),
                            _r(at[:, k, :]),
                            start=(k == 0),
                            stop=(k == PAIRS - 1),
                        )
                    ys = y_pool.tile([128, FCW], F32, tag="y")
                    nc.vector.tensor_copy(out=ys[:], in_=pp[:])
                    nc.sync.dma_start(out=yT_r[m, :, ts(fc, FCW)], in_=ys[:])

            # proj of f-chunk N is emitted during f-chunk N+1's second pair
            # so its attnT dependency (the normalize chain) resolves while
            # the PE streams the next chunk's scores -- no idle, no HAM
            # re-throttle before the projection burst.
            pending = None
            for fc in range(FC):
                at = at_pool.tile([128, PAIRS, FCW], F32R, tag="at")
                atraw = ar_pool.tile([128, PAIRS, FCW], F32, tag="ar")
                dens = d_pool.tile([HPC, FCW], F32, tag="dn", name=f"dens{fc}")
                for pj in range(PAIRS):
                    emit_scores_exp_pv(fc, pj, atraw, dens)
                    if pj == 1 and pending is not None:
                        emit_proj(*pending)
                        pending = None
                emit_normalize(fc, at, atraw, dens)
                pending = (fc, at)
            emit_proj(*pending)

    nc.compile()
    return nc


def shard_inputs(x, w_attn, w_proj):
    """Build the 8 per-core input maps from full inputs."""
    x = np.asarray(x, dtype=np.float32)
    w_attn = np.asarray(w_attn, dtype=np.float32)
    w_proj = np.asarray(w_proj, dtype=np.float32)
    in_maps = []
    for c in range(NCORES):
        b, g = divmod(c, 2)
        cols = slice(512 * g, 512 * (g + 1))
        wq = w_attn[:, 0:D][:, cols]
        wk = w_attn[:, D : 2 * D][:, cols]
        wv = w_attn[:, 2 * D : 3 * D][:, cols]
        in_maps.append(
            {
                "xT": np.ascontiguousarray(x[b].T),
                "wqk": np.ascontiguousarray(np.concatenate([wq, wk], axis=1)),
                "wv": np.ascontiguousarray(wv),
                "wproj": np.ascontiguousarray(w_proj[cols, :]),
                "vones": np.ones((128, 128), dtype=np.float32),
            }
        )
    return in_maps


def kernel(x, attention_mask, w_attn, b_attn, w_proj, b_proj):
    global LAST_RESULTS
    from concourse.bass_utils import run_bass_kernel_spmd

    if "nc" not in _COMPILED:
        _COMPILED["nc"] = build_nc()
    nc = _COMPILED["nc"]

    in_maps = shard_inputs(x, w_attn, w_proj)
    trace = os.environ.get("KERNEL_TRACE", "0") == "1"
    res = run_bass_kernel_spmd(
        nc, in_maps, core_ids=list(range(NCORES)), trace=trace
    )
    LAST_RESULTS = res

    b_attn = np.asarray(b_attn, dtype=np.float32)
    b_proj = np.asarray(b_proj, dtype=np.float32)
    # b_attn is structurally zero in this problem; the kernel ignores it.
    y = np.empty((B, S, D), dtype=np.float32)
    for b in range(B):
        yT = res.results[2 * b]["yT"] + res.results[2 * b + 1]["yT"]
        y[b] = yT.T + b_proj
    return y
